# revision 12
# baseline (speedup 1.0000x reference)
"""Trainium2 Bass kernel for nn_MixAttention_v2 (B=16, N=4096, M=64, C=192, H=6).

Sharding: data-parallel over B across 8 NeuronCores (2 batches per core),
SPMD single program, per-core input shards fed via run_bass_kernel_spmd.

Device dataflow (per batch, fully fused, one pass over N):
  init:   kv2T = [Wkv2^T;bkv2]^T-chunks @ cT_ext          (k^T, v2^T per head)
          KW[c,(h,m)]   = Wq_h^T @ k_h^T  (+BS row from bq)  -- Wq folded into k
          V2comb[(h,m),j] = v2_h @ WpxT-rows                 -- Wpx folded into v2
  tile t: S[n,(h,m)] = [x|1]^T-chunks @ [KW;BS]              (2 matmuls, K=128+65)
          E_x = exp(scale_x*S)  (ScalarE, bf16)
          E_c = E_x*E_x         (exact: scale_c == 2*scale_x)
          r_h[n] = rowsum(E_x); ax = E_x / r                 (softmax over m)
          T_pair += E_c_pair^T @ [x|1]    (PSUM-accumulated over all 32 tiles;
                                           col 192 accumulates l = sum_n E_c)
          axT = PE-transpose(ax);  out_px = sum_p axT_p^T @ V2comb_p
  epilog: Tn = T * (1/l);  out_c = sum_h Tn_h @ WcombT_h     (Wpc@Wv1 host-folded)

Host folds (exact): bpx added post-hoc; bv1/bpc folded into out_c constant
(softmax rows sum to 1); weights pre-transposed and pre-combined in numpy.
All PE operands bf16, all accumulation fp32 in PSUM.
"""
import math
import os
import sys

import numpy as np

if "/opt/trn_rl_repo" not in sys.path:
    sys.path.insert(0, "/opt/trn_rl_repo")

import ml_dtypes  # noqa: E402

import concourse.bass as bass  # noqa: E402
import concourse.bacc as bacc  # noqa: E402
import concourse.mybir as mybir  # noqa: E402
import concourse.tile as tile  # noqa: E402

BF16NP = ml_dtypes.bfloat16
F32 = mybir.dt.float32
BF16 = mybir.dt.bfloat16

B, N, M, C, H = 16, 4096, 64, 192, 6
D = C // H
NCORES = 8
BL = B // NCORES       # batches per core
NT = N // 128          # 128-token tiles per batch
NBLK = N // 512        # dma blocks per batch
SCALE = C ** -0.5
SCALE_X = math.log(M, N) * SCALE
SCALE_C = SCALE
assert abs(SCALE_C - 2.0 * SCALE_X) < 1e-18

Exp = mybir.ActivationFunctionType.Exp
AX = mybir.AxisListType.X
ADD = mybir.AluOpType.add


def ts(i, size):
    return slice(i * size, (i + 1) * size)


def _emit(nc: bass.Bass, tc: "tile.TileContext", ctx, aps):
    const = ctx.enter_context(tc.tile_pool(name="const", bufs=1))
    bpool = ctx.enter_context(tc.tile_pool(name="batch", bufs=2))
    xtp = ctx.enter_context(tc.tile_pool(name="xt", bufs=2))
    workp = ctx.enter_context(tc.tile_pool(name="work", bufs=2))
    # PSUM pools: 2(S) + 1(axT) + 1(px) + 3(T) + 1(scratch) = 8 banks
    pss = ctx.enter_context(tc.tile_pool(name="ps_s", bufs=2, space="PSUM"))
    paxp = ctx.enter_context(tc.tile_pool(name="ps_axt", bufs=1, space="PSUM"))
    ppxp = ctx.enter_context(tc.tile_pool(name="ps_px", bufs=1, space="PSUM"))
    psT = ctx.enter_context(tc.tile_pool(name="ps_T", bufs=1, space="PSUM"))
    psc = ctx.enter_context(tc.tile_pool(name="ps_scr", bufs=1, space="PSUM"))

    # ---- constants (once per core) ----
    ident = const.tile([128, 128], BF16)
    nc.sync.dma_start(ident[:], aps["identity"][:, :])
    wq_a = const.tile([128, C], BF16)
    nc.sync.dma_start(wq_a[:], aps["wq_nat"][0:128, :])
    wq_b = const.tile([64, C], BF16)
    nc.sync.dma_start(wq_b[:], aps["wq_nat"][128:192, :])
    bq_a = const.tile([128, 1], BF16)
    nc.sync.dma_start(bq_a[:], aps["bq_col"][0:128, :])
    bq_b = const.tile([64, 1], BF16)
    nc.sync.dma_start(bq_b[:], aps["bq_col"][128:192, :])
    wkv_a = const.tile([128, 2 * C], BF16)
    nc.sync.dma_start(wkv_a[:], aps["wkvT_ext"][0:128, :])
    wkv_b = const.tile([65, 2 * C], BF16)
    nc.sync.dma_start(wkv_b[:], aps["wkvT_ext"][128:193, :])
    wpx_a = const.tile([128, C], BF16)
    nc.sync.dma_start(wpx_a[:], aps["wpxT"][0:128, :])
    wpx_b = const.tile([64, C], BF16)
    nc.sync.dma_start(wpx_b[:], aps["wpxT"][128:192, :])
    wc_a = const.tile([96, H * C], BF16)
    nc.sync.dma_start(wc_a[:], aps["wcombT"][0:96, :])
    wc_b = const.tile([96, H * C], BF16)
    nc.sync.dma_start(wc_b[:], aps["wcombT"][96:192, :])

    for b in range(BL):
        # ================= batch init =================
        ct_a = bpool.tile([128, M], BF16, tag="ct_a")
        nc.sync.dma_start(ct_a[:], aps["cT_ext"][b, 0:128, :])
        ct_b = bpool.tile([65, M], BF16, tag="ct_b")
        nc.sync.dma_start(ct_b[:], aps["cT_ext"][b, 128:193, :])

        # Block-diagonal tiles: KB[(h,d),(h,m)] = kT_h on blocks, zeros else;
        # VB same for v2T_h.  (matmul operands must share base partition 0,
        # so per-head contractions are expressed as block-diag full-K mms.)
        kb_a = bpool.tile([128, H * M], BF16, tag="kb_a", name=f"kba{b}")
        kb_b = bpool.tile([64, H * M], BF16, tag="kb_b", name=f"kbb{b}")
        vb_a = bpool.tile([128, H * M], BF16, tag="vb_a", name=f"vba{b}")
        vb_b = bpool.tile([64, H * M], BF16, tag="vb_b", name=f"vbb{b}")
        for t_ in (kb_a, kb_b, vb_a, vb_b):
            nc.vector.memset(t_[:], 0.0)

        def kb(h):  # block slot for head h in (kb_a|kb_b)
            return kb_a[ts(h, D), ts(h, M)] if h < 4 else kb_b[ts(h - 4, D), ts(h, M)]

        def vb(h):
            return vb_a[ts(h, D), ts(h, M)] if h < 4 else vb_b[ts(h - 4, D), ts(h, M)]

        # kv2T chunks: rows 0:128 / 128:256 / 256:384 of [k^T; v2^T], scattered
        # straight into the block-diag tiles.
        for o in range(3):
            pk = psc.tile([128, M], F32, tag="scratch", name=f"pkv{b}{o}")
            nc.tensor.matmul(pk[:], wkv_a[:, ts(o, 128)], ct_a[:], start=True, stop=False)
            nc.tensor.matmul(pk[:], wkv_b[:, ts(o, 128)], ct_b[:], start=False, stop=True)
            for s in range(4):  # rows 32s:32s+32 of this chunk = (h,d) rows 128o+32s
                row = 128 * o + 32 * s
                if row < C:
                    nc.vector.tensor_copy(kb(row // D), pk[ts(s, D), :])
                else:
                    nc.vector.tensor_copy(vb((row - C) // D), pk[ts(s, D), :])

        # KW[c,(h,m)] = Wq^T-blockdiag contraction; BS bias row from bq
        pkw_a = psc.tile([128, H * M], F32, tag="scratch", name=f"pkwa{b}")
        nc.tensor.matmul(pkw_a[:], wq_a[:, 0:128], kb_a[:], start=True, stop=False)
        nc.tensor.matmul(pkw_a[:], wq_b[:, 0:128], kb_b[:], start=False, stop=True)
        kw_a = bpool.tile([128, H * M], BF16, tag="kw_a", name=f"kwa{b}")
        nc.vector.tensor_copy(kw_a[:], pkw_a[:])

        pkw_b = psc.tile([64, H * M], F32, tag="scratch", name=f"pkwb{b}")
        nc.tensor.matmul(pkw_b[:], wq_a[:, 128:192], kb_a[:], start=True, stop=False)
        nc.tensor.matmul(pkw_b[:], wq_b[:, 128:192], kb_b[:], start=False, stop=True)
        pbs = psc.tile([1, H * M], F32, tag="scratch", name=f"pbs{b}")
        nc.tensor.matmul(pbs[:], bq_a[:], kb_a[:], start=True, stop=False)
        nc.tensor.matmul(pbs[:], bq_b[:], kb_b[:], start=False, stop=True)
        kw_b = bpool.tile([65, H * M], BF16, tag="kw_b", name=f"kwb{b}")
        nc.vector.tensor_copy(kw_b[0:64, :], pkw_b[:])
        nc.vector.tensor_copy(kw_b[64:65, :], pbs[:])

        # V2comb chunks [(h,m) 128-rows, C] via VB-blockdiag @ WpxT
        v2c_sb = []
        for p in range(3):
            pv = psc.tile([128, C], F32, tag="scratch", name=f"pv2c{b}{p}")
            nc.tensor.matmul(pv[:], vb_a[:, ts(p, 128)], wpx_a[:], start=True, stop=False)
            nc.tensor.matmul(pv[:], vb_b[:, ts(p, 128)], wpx_b[:], start=False, stop=True)
            vt = bpool.tile([128, C], BF16, tag=f"v2c{p}", name=f"v2c{b}{p}")
            nc.vector.tensor_copy(vt[:], pv[:])
            v2c_sb.append(vt)

        # T accumulators (persist across the batch; one bank each — PSUM
        # accumulation groups must not share a zero region/bank)
        pT = [psT.tile([128, C + 1], F32, tag=f"T{p}", name=f"T{p}_{b}")
              for p in range(3)]

        def Tacc(p):
            return pT[p][:]

        # ================= steady tiles =================
        xt_a = xt_b = xn = None
        for t in range(NT):
            if t % 4 == 0:
                j = t // 4
                xt_a = xtp.tile([128, 512], BF16, tag="xta", name=f"xta{b}{j}")
                nc.sync.dma_start(xt_a[:], aps["xT_ext"][b, 0:128, ts(j, 512)])
                xt_b = xtp.tile([65, 512], BF16, tag="xtb", name=f"xtb{b}{j}")
                nc.sync.dma_start(xt_b[:], aps["xT_ext"][b, 128:193, ts(j, 512)])
                xn = xtp.tile([128, 4 * (C + 1)], BF16, tag="xn", name=f"xn{b}{j}")
                nc.sync.dma_start(
                    xn[:].rearrange("p (a c) -> p a c", a=4),
                    aps["x_ext"][b, ts(j, 512), :].rearrange("(a p) c -> p a c", p=128),
                )
            tt = t % 4

            ps = pss.tile([128, H * M], F32, tag="S", name=f"S{b}_{t}")
            nc.tensor.matmul(ps[:], xt_a[:, ts(tt, 128)], kw_a[:], start=True, stop=False)
            nc.tensor.matmul(ps[:], xt_b[:, ts(tt, 128)], kw_b[:], start=False, stop=True)

            ex = workp.tile([128, H * M], BF16, tag="ex", name=f"ex{b}_{t}")
            nc.scalar.activation(ex[:], ps[:], Exp, scale=float(SCALE_X))

            ec = workp.tile([128, H * M], BF16, tag="ec", name=f"ec{b}_{t}")
            nc.vector.tensor_mul(ec[:], ex[:], ex[:])

            r = workp.tile([128, H], F32, tag="r", name=f"r{b}_{t}")
            nc.vector.tensor_reduce(
                r[:], ex[:].rearrange("p (h m) -> p h m", h=H), axis=AX, op=ADD)
            rinv = workp.tile([128, H], F32, tag="rinv", name=f"rinv{b}_{t}")
            nc.vector.reciprocal(rinv[:], r[:])
            ax = workp.tile([128, H * M], BF16, tag="ax", name=f"ax{b}_{t}")
            for h in range(H):
                nc.vector.tensor_scalar_mul(ax[:, ts(h, M)], ex[:, ts(h, M)],
                                            rinv[:, h:h + 1])

            xn_v = xn[:].rearrange("p (a c) -> p a c", a=4)[:, tt, :]  # [128, 193]
            for p in range(3):
                nc.tensor.matmul(Tacc(p), ec[:, ts(p, 128)], xn_v,
                                 start=(t == 0), stop=(t == NT - 1))

            paxT = paxp.tile([128, H * M], BF16, tag="axT", name=f"paxT{b}_{t}")
            for p in range(3):
                nc.tensor.transpose(paxT[:, ts(p, 128)], ax[:, ts(p, 128)], ident[:])
            axs = workp.tile([128, H * M], BF16, tag="axs", name=f"axs{b}_{t}")
            nc.vector.tensor_copy(axs[:], paxT[:])

            ppx = ppxp.tile([128, C], F32, tag="px", name=f"ppx{b}_{t}")
            for p in range(3):
                nc.tensor.matmul(ppx[:], axs[:, ts(p, 128)], v2c_sb[p][:],
                                 start=(p == 0), stop=(p == 2))
            oxs = workp.tile([128, C], F32, tag="oxs", name=f"oxs{b}_{t}")
            nc.vector.tensor_copy(oxs[:], ppx[:])
            nc.sync.dma_start(aps["out_x"][b, ts(t, 128), :], oxs[:])

        # ================= batch epilogue =================
        # Both c-halves of Tn^T share one bank: [96, (hf, p, 2h*m)] bf16
        pTnT = psc.tile([96, 2 * H * M], BF16, tag="scratch", name=f"pTnT{b}")
        for p in range(3):
            li = bpool.tile([128, 1], F32, tag=f"li{p}", name=f"li{b}{p}")
            nc.vector.reciprocal(li[:], Tacc(p)[:, C:C + 1])
            tn = bpool.tile([128, C], BF16, tag=f"tn{p}", name=f"tn{b}{p}")
            nc.vector.tensor_scalar_mul(tn[:], Tacc(p)[:, 0:C], li[:, 0:1])
            for hf in range(2):
                nc.tensor.transpose(pTnT[:, hf * H * M + 128 * p:hf * H * M + 128 * (p + 1)],
                                    tn[:, ts(hf, 96)], ident[:])
        tnT = bpool.tile([96, 2 * H * M], BF16, tag="tnT", name=f"tnT{b}")
        nc.vector.tensor_copy(tnT[:], pTnT[:])
        poc = psc.tile([64, C], F32, tag="scratch", name=f"poc{b}")
        for h in range(H):
            p, e = h // 2, h % 2
            for hf in range(2):
                off = hf * H * M + p * 128 + e * M
                nc.tensor.matmul(poc[:], tnT[:, off:off + M],
                                 wc_a[:, ts(h, C)] if hf == 0 else wc_b[:, ts(h, C)],
                                 start=(h == 0 and hf == 0), stop=(h == H - 1 and hf == 1))
        ocs = bpool.tile([64, C], F32, tag="ocs", name=f"ocs{b}")
        nc.vector.tensor_copy(ocs[:], poc[:])
        nc.sync.dma_start(aps["out_c"][b, :, :], ocs[:])


def build_program():
    nc = bacc.Bacc("TRN2", target_bir_lowering=False, debug=False,
                   num_devices=NCORES)
    aps = {
        "x_ext": nc.dram_tensor("x_ext", [BL, N, C + 1], BF16, kind="ExternalInput").ap(),
        "xT_ext": nc.dram_tensor("xT_ext", [BL, C + 1, N], BF16, kind="ExternalInput").ap(),
        "cT_ext": nc.dram_tensor("cT_ext", [BL, C + 1, M], BF16, kind="ExternalInput").ap(),
        "wkvT_ext": nc.dram_tensor("wkvT_ext", [C + 1, 2 * C], BF16, kind="ExternalInput").ap(),
        "wq_nat": nc.dram_tensor("wq_nat", [C, C], BF16, kind="ExternalInput").ap(),
        "bq_col": nc.dram_tensor("bq_col", [C, 1], BF16, kind="ExternalInput").ap(),
        "wpxT": nc.dram_tensor("wpxT", [C, C], BF16, kind="ExternalInput").ap(),
        "wcombT": nc.dram_tensor("wcombT", [C, H * C], BF16, kind="ExternalInput").ap(),
        "identity": nc.dram_tensor("identity", [128, 128], BF16, kind="ExternalInput").ap(),
        "out_x": nc.dram_tensor("out_x", [BL, N, C], F32, kind="ExternalOutput").ap(),
        "out_c": nc.dram_tensor("out_c", [BL, M, C], F32, kind="ExternalOutput").ap(),
    }
    from contextlib import ExitStack
    with tile.TileContext(nc) as tc:
        with ExitStack() as ctx:
            _emit(nc, tc, ctx, aps)
    nc.compile()
    return nc


def host_prep(inputs):
    """Build the per-core in_maps (host-side numpy prep) and host-fold consts."""
    x = np.asarray(inputs["x"], np.float32)
    c = np.asarray(inputs["c"], np.float32)
    Wqv1 = np.asarray(inputs["Wqv1"], np.float32)
    bqv1 = np.asarray(inputs["bqv1"], np.float32)
    Wkv2 = np.asarray(inputs["Wkv2"], np.float32)
    bkv2 = np.asarray(inputs["bkv2"], np.float32)
    Wpx = np.asarray(inputs["Wpx"], np.float32)
    bpx = np.asarray(inputs["bpx"], np.float32)
    Wpc = np.asarray(inputs["Wpc"], np.float32)
    bpc = np.asarray(inputs["bpc"], np.float32)

    x_ext = np.concatenate([x, np.ones((B, N, 1), np.float32)], axis=2).astype(BF16NP)
    xT_ext = np.concatenate(
        [np.ascontiguousarray(x.transpose(0, 2, 1)), np.ones((B, 1, N), np.float32)],
        axis=1).astype(BF16NP)
    cT_ext = np.concatenate(
        [np.ascontiguousarray(c.transpose(0, 2, 1)), np.ones((B, 1, M), np.float32)],
        axis=1).astype(BF16NP)
    wkvT_ext = np.concatenate([Wkv2.T, bkv2[None, :]], axis=0).astype(BF16NP)
    wq_nat = Wqv1[:C].astype(BF16NP)
    bq_col = bqv1[:C][:, None].astype(BF16NP)
    wpxT = Wpx.T.astype(BF16NP)
    wcombT = np.zeros((C, H * C), np.float32)
    for h in range(H):
        wcombT[:, ts(h, C)] = (Wpc[:, ts(h, D)] @ Wqv1[C + h * D:C + (h + 1) * D, :]).T
    wcombT = wcombT.astype(BF16NP)
    identity = np.eye(128, dtype=BF16NP)

    in_maps = []
    for core in range(NCORES):
        bs = slice(core * BL, (core + 1) * BL)
        in_maps.append({
            "x_ext": np.ascontiguousarray(x_ext[bs]),
            "xT_ext": np.ascontiguousarray(xT_ext[bs]),
            "cT_ext": np.ascontiguousarray(cT_ext[bs]),
            "wkvT_ext": wkvT_ext,
            "wq_nat": wq_nat,
            "bq_col": bq_col,
            "wpxT": wpxT,
            "wcombT": wcombT,
            "identity": identity,
        })
    out_x_bias = bpx
    out_c_bias = Wpc @ bqv1[C:] + bpc
    return in_maps, out_x_bias, out_c_bias


def host_post(results, out_x_bias, out_c_bias):
    out_x = np.concatenate([np.asarray(results[i]["out_x"], np.float32)
                            for i in range(NCORES)], axis=0)
    out_c = np.concatenate([np.asarray(results[i]["out_c"], np.float32)
                            for i in range(NCORES)], axis=0)
    out_x += out_x_bias[None, None, :]
    out_c += out_c_bias[None, None, :]
    return out_x, out_c


_NC_CACHE = None


def _get_program():
    global _NC_CACHE
    if _NC_CACHE is None:
        _NC_CACHE = build_program()
    return _NC_CACHE


def kernel(**inputs):
    from concourse.bass_utils import run_bass_kernel_spmd
    nc = _get_program()
    in_maps, bx, bc = host_prep(inputs)
    res = run_bass_kernel_spmd(nc, in_maps, list(range(NCORES)))
    return host_post(res.results, bx, bc)
